# revision 1
# baseline (speedup 1.0000x reference)
"""Multi-head attention (B=2, S=2048, D=1024, H=16, causal) on 8 TRN2 NeuronCores.

Sharding: 8 cores = 2 batches x 4 head-groups (4 heads each).  Each core
computes the QKV projections for its head slice, causal attention for its 4
heads, and the partial output projection (input-dim slice of Wo).  The
all-reduce over head-groups happens at gather time on the host (sum of 4
partials per batch), which is the "all-reduce after the output projection"
of a tensor-parallel split.

Everything on device works in token-transposed layout ([feature, token]) so
no on-device transposes are needed:
  scores^T[kv, q] = K_projT_tile^T @ Q_projT   (K = dh = 64)
  P = exp(scores^T)  (no max subtraction needed: scores ~ N(0,1), |s| < ~7)
  out^T[dh(+1), q] = [V | ones]^T @ P          (ones column -> softmax denom)
  partial^T[dmodel, tok] = WoT_slice^T @ attn_out^T

Perf notes:
  - causal trimming: score matmul / exp / AV only cover valid q columns of
    diagonal kv-tiles; the per-tile mask multiply shrinks to one 128x128
    causal block.
  - score matmuls have K=64 (half the PE rows): odd kv-tiles are issued at
    tile rows 64-127 (via partition-swapped copies of Q/K projections) so
    adjacent score matmuls run concurrently in the PE array.
  - O-projection accumulates in two half-groups (heads 0+1, heads 2+3) so
    the heads-0/1 half runs on the PE while heads 2/3 attention is still
    ACT-bound; the halves are summed in the output copy.
"""

import math
import os

import numpy as np
import ml_dtypes

_BF16 = ml_dtypes.bfloat16

B, S, D = 2, 2048, 1024
H, DH = 16, 64
NCORES = 8
GRP = 4  # heads per core
KT = D // 128  # 8 k-tiles over d_model
NQ = 512  # q tile width (free dim of score tiles)
QTILES = S // NQ  # 4
KVTILES = S // 128  # 16

last_results = None

_programs = {}


def _build_program(causal: bool):
    OPT_INLINE = os.environ.get("KOPT_INLINE", "1") == "1"
    OPT_OPACK = os.environ.get("KOPT_OPACK", "1") == "1"
    OPT_SCADJ = os.environ.get("KOPT_SCADJ", "1") == "1"

    import concourse.bass as bass
    import concourse.mybir as mybir
    import concourse.tile as tile
    from concourse import bacc

    f32 = mybir.dt.float32
    bf16 = mybir.dt.bfloat16
    Exp = mybir.ActivationFunctionType.Exp
    Copy = mybir.ActivationFunctionType.Copy

    nc = bacc.Bacc(
        "TRN2",
        target_bir_lowering=False,
        debug=False,
        enable_asserts=False,
        num_devices=NCORES,
    )

    qT = nc.dram_tensor("qT", [D, S], bf16, kind="ExternalInput").ap()
    kT = nc.dram_tensor("kT", [D, S], bf16, kind="ExternalInput").ap()
    vT = nc.dram_tensor("vT", [D, S], bf16, kind="ExternalInput").ap()
    wqT = nc.dram_tensor("wqT", [D, 256], bf16, kind="ExternalInput").ap()
    wkT = nc.dram_tensor("wkT", [D, 256], bf16, kind="ExternalInput").ap()
    wvT = nc.dram_tensor("wvT", [D, 256], bf16, kind="ExternalInput").ap()
    woT = nc.dram_tensor("woT", [256, D], bf16, kind="ExternalInput").ap()
    if not causal:
        maskT = nc.dram_tensor("maskT", [S, S], bf16, kind="ExternalInput").ap()
    out = nc.dram_tensor("out", [D, S], f32, kind="ExternalOutput").ap()

    with tile.TileContext(nc) as tc:
        with (
            tc.tile_pool(name="persist", bufs=1) as sb,
            tc.tile_pool(name="stream", bufs=8) as stream,
            tc.tile_pool(name="psum", bufs=1, space="PSUM") as psum,
            tc.tile_pool(name="p_sb", bufs=6) as pbuf,
            tc.tile_pool(name="r_sb", bufs=4) as rpool,
            tc.tile_pool(name="m_sb", bufs=4) as mpool,
            tc.tile_pool(name="o_sb", bufs=4) as opool,
        ):
            # ---- persistent SBUF tensors ----
            wq_sb = sb.tile([128, KT, 256], bf16)
            wk_sb = sb.tile([128, KT, 256], bf16)
            wv_sb = sb.tile([128, KT, 256], bf16)
            wo2 = sb.tile([128, 2, D], bf16)  # head h at rows 64*(h%2), chunk h//2
            qproj = sb.tile([128, 2, S], bf16)
            kproj = sb.tile([128, 2, S], bf16)
            vproj = sb.tile([128, KVTILES, GRP, 66], bf16)
            attn2 = sb.tile([128, 2, S], bf16)  # head h at rows 64*(h%2), chunk h//2

            # input DMAs: q first (starts the pipeline); k-side on the
            # software-DGE queues (gpsimd) to double input bandwidth
            qts, kts, vts = [], [], []
            for kt in range(KT):
                nc.sync.dma_start(wq_sb[:, kt, :], wqT[128 * kt : 128 * kt + 128, :])
            for kt in range(KT):
                t = stream.tile([128, S], bf16, tag="qTt", bufs=8)
                nc.sync.dma_start(t[:], qT[128 * kt : 128 * kt + 128, :])
                qts.append(t)
            for kt in range(KT):
                nc.gpsimd.dma_start(wk_sb[:, kt, :], wkT[128 * kt : 128 * kt + 128, :])
            for kt in range(KT):
                t = stream.tile([128, S], bf16, tag="kTt", bufs=8)
                nc.gpsimd.dma_start(t[:], kT[128 * kt : 128 * kt + 128, :])
                kts.append(t)
            for kt in range(KT):
                nc.gpsimd.dma_start(wv_sb[:, kt, :], wvT[128 * kt : 128 * kt + 128, :])
            for kt in range(KT):
                t = stream.tile([128, S], bf16, tag="vTt", bufs=8)
                nc.sync.dma_start(t[:], vT[128 * kt : 128 * kt + 128, :])
                vts.append(t)
            for h in range(GRP):
                base = 64 * (h % 2)
                nc.sync.dma_start(
                    wo2[base : base + 64, h // 2, :], woT[64 * h : 64 * h + 64, :]
                )

            # ones columns at index 0 and 65 of vproj (V lands in cols 1..64)
            nc.gpsimd.memset(vproj[:], 1.0)

            if causal:
                # single 128x128 causal block: keep where q_local >= kv_local
                mask128 = sb.tile([128, 128], bf16)
                nc.gpsimd.memset(mask128[:], 1.0)
                nc.gpsimd.affine_select(
                    out=mask128[:],
                    in_=mask128[:],
                    compare_op=mybir.AluOpType.is_ge,
                    fill=0.0,
                    base=0,
                    pattern=[[1, 128]],
                    channel_multiplier=-1,
                )

            def qkproj(which, m2, ns):
                w_sb = wq_sb if which == "q" else wk_sb
                xt = qts if which == "q" else kts
                proj = qproj if which == "q" else kproj
                for n in ns:
                    ps = psum.tile([128, NQ], f32, tag="sc", bufs=4)
                    for kt in range(KT):
                        nc.tensor.matmul(
                            ps[:],
                            w_sb[:, kt, 128 * m2 : 128 * m2 + 128],
                            xt[kt][:, NQ * n : NQ * n + NQ],
                            start=(kt == 0),
                            stop=(kt == KT - 1),
                        )
                    nc.vector.tensor_copy(proj[:, m2, NQ * n : NQ * n + NQ], ps[:])

            def vproj_tiles(mts):
                for mt in mts:
                    ps = psum.tile([128, 256], f32, tag="sc", bufs=4)
                    for kt in range(KT):
                        nc.tensor.matmul(
                            ps[:],
                            vts[kt][:, 128 * mt : 128 * mt + 128],
                            wv_sb[:, kt, :],
                            start=(kt == 0),
                            stop=(kt == KT - 1),
                        )
                    nc.vector.tensor_copy(
                        vproj[:, mt, :, 1:65],
                        ps[:].rearrange("p (h d) -> p h d", h=GRP),
                    )

            def attn_pair(c2, j):
                # heads (2*c2, 2*c2+1) together: the even head lives at rows
                # 0-63 and the odd head at rows 64-127 of projection chunk c2,
                # so their score matmuls are emitted adjacently and run
                # concurrently in the two row halves of the PE array.
                avs = [
                    psum.tile([65, NQ], f32, tag="av", bufs=2, name=f"av{c2}{j}{i}")
                    for i in range(2)
                ]
                ktiles = 4 * j + 4 if causal else KVTILES

                def off_of(t):
                    d = t - 4 * j
                    return 128 * d if (causal and d >= 0) else 0

                for t in range(ktiles):
                    off = off_of(t)
                    pps = []
                    sps = []
                    for i in range(2):
                        base = 64 * i
                        sp = psum.tile([128, NQ], f32, tag="sc", bufs=4)
                        nc.tensor.matmul(
                            sp[:, off:NQ],
                            kproj[base : base + 64, c2, 128 * t : 128 * t + 128],
                            qproj[base : base + 64, c2, NQ * j + off : NQ * j + NQ],
                            start=True,
                            stop=True,
                        )
                        sps.append(sp)
                    for i in range(2):
                        p = pbuf.tile([128, NQ], bf16, tag="p")
                        nc.scalar.activation(p[:, off:NQ], sps[i][:, off:NQ], Exp)
                        pps.append(p)
                    for i in range(2):
                        if causal:
                            if t - 4 * j >= 0:
                                nc.vector.tensor_mul(
                                    pps[i][:, off : off + 128],
                                    pps[i][:, off : off + 128],
                                    mask128[:],
                                )
                        else:
                            mt_t = mpool.tile([128, NQ], bf16, tag="mt")
                            nc.sync.dma_start(
                                mt_t[:],
                                maskT[128 * t : 128 * t + 128, NQ * j : NQ * j + NQ],
                            )
                            nc.vector.tensor_mul(pps[i][:], pps[i][:], mt_t[:])
                    for i in range(2):
                        nc.tensor.matmul(
                            avs[i][:, off:NQ],
                            vproj[:, t, 2 * c2 + i, 1:66],
                            pps[i][:, off:NQ],
                            start=(t == 0),
                            stop=(t == ktiles - 1),
                        )
                # normalize: attn2[rows, c2, q] = av[0:64, q] / av[64, q]
                for i in range(2):
                    av = avs[i]
                    rs = rpool.tile([65, NQ], f32, tag="rs")
                    nc.vector.tensor_copy(rs[64:65, :], av[64:65, :])
                    rq = rpool.tile([128, 4], f32, tag="rq")
                    nc.sync.dma_start(rq[:], rs[64:65, :])
                    rqr = rpool.tile([128, 4], f32, tag="rqr")
                    nc.vector.reciprocal(rqr[:], rq[:])
                    rr = rpool.tile([1, NQ], f32, tag="rr")
                    nc.sync.dma_start(rr[:], rqr[:])
                    rb = rpool.tile([64, NQ], f32, tag="rb")
                    nc.gpsimd.partition_broadcast(rb[:], rr[0:1, :], channels=64)
                    if i == 0:
                        nc.vector.tensor_mul(
                            attn2[0:64, c2, NQ * j : NQ * j + NQ], av[0:64, :], rb[:]
                        )
                    else:
                        tmpn = rpool.tile([64, NQ], bf16, tag="tmpn")
                        nc.vector.tensor_mul(tmpn[:], av[0:64, :], rb[:])
                        nc.sync.dma_start(
                            attn2[64:128, c2, NQ * j : NQ * j + NQ], tmpn[:]
                        )

            def oproj_n(n):
                for m in range(D // 128):
                    ps = psum.tile([128, NQ], f32, tag="op", bufs=2)
                    # head pairs stacked in partition halves -> K=128 contracts
                    # two heads per matmul
                    for c2 in range(2):
                        nc.tensor.matmul(
                            ps[:],
                            wo2[:, c2, 128 * m : 128 * m + 128],
                            attn2[:, c2, NQ * n : NQ * n + NQ],
                            start=(c2 == 0),
                            stop=(c2 == 1),
                        )
                    ot = opool.tile([128, NQ], f32, tag="ot")
                    nc.vector.tensor_copy(ot[:], ps[:])
                    nc.sync.dma_start(
                        out[128 * m : 128 * m + 128, NQ * n : NQ * n + NQ], ot[:]
                    )

            # ---- emission order: keep the PE dense ----
            qkproj("q", 0, range(QTILES))
            qkproj("k", 0, range(QTILES))
            vproj_tiles(range(0, 4))
            attn_pair(0, 0)
            qkproj("q", 1, [0, 1])
            vproj_tiles(range(4, 8))
            attn_pair(0, 1)
            qkproj("q", 1, [2, 3])
            qkproj("k", 1, [0, 1])
            vproj_tiles(range(8, 12))
            attn_pair(0, 2)
            qkproj("k", 1, [2, 3])
            vproj_tiles(range(12, 16))
            attn_pair(0, 3)
            for j in range(QTILES):
                attn_pair(1, j)
                oproj_n(j)

    nc.compile()
    return nc


def _get_program(causal: bool):
    if causal not in _programs:
        _programs[causal] = _build_program(causal)
    return _programs[causal]


def kernel(query, key, value, mask, Wq, Wk, Wv, Wo):
    global last_results
    from concourse.bass_utils import run_bass_kernel_spmd

    query = np.asarray(query, dtype=np.float32)
    key = np.asarray(key, dtype=np.float32)
    value = np.asarray(value, dtype=np.float32)
    Wq = np.asarray(Wq, dtype=np.float32)
    Wk = np.asarray(Wk, dtype=np.float32)
    Wv = np.asarray(Wv, dtype=np.float32)
    Wo = np.asarray(Wo, dtype=np.float32)
    m2d = np.asarray(mask).reshape(S, S).astype(bool)

    causal = bool(np.array_equal(m2d, np.tril(np.ones((S, S), dtype=bool))))
    nc = _get_program(causal)

    scale = 1.0 / math.sqrt(DH)
    WqT = np.ascontiguousarray((Wq * scale).T).astype(_BF16)
    WkT = np.ascontiguousarray(Wk.T).astype(_BF16)
    WvT = np.ascontiguousarray(Wv.T).astype(_BF16)
    WoT = np.ascontiguousarray(Wo.T).astype(_BF16)
    xT = {
        "qT": [np.ascontiguousarray(query[b].T).astype(_BF16) for b in range(B)],
        "kT": [np.ascontiguousarray(key[b].T).astype(_BF16) for b in range(B)],
        "vT": [np.ascontiguousarray(value[b].T).astype(_BF16) for b in range(B)],
    }
    if not causal:
        maskT = np.ascontiguousarray(m2d.T).astype(_BF16)

    in_maps = []
    for c in range(NCORES):
        b, g = c // 4, c % 4
        sl = slice(256 * g, 256 * g + 256)
        im = {
            "qT": xT["qT"][b],
            "kT": xT["kT"][b],
            "vT": xT["vT"][b],
            "wqT": np.ascontiguousarray(WqT[:, sl]),
            "wkT": np.ascontiguousarray(WkT[:, sl]),
            "wvT": np.ascontiguousarray(WvT[:, sl]),
            "woT": np.ascontiguousarray(WoT[sl, :]),
        }
        if not causal:
            im["maskT"] = maskT
        in_maps.append(im)

    trace = os.environ.get("KERNEL_PROFILE", "") == "1"
    res = run_bass_kernel_spmd(nc, in_maps, list(range(NCORES)), trace=trace)
    last_results = res

    outp = np.empty((B, S, D), dtype=np.float32)
    for b in range(B):
        acc = res.results[4 * b]["out"].astype(np.float32)
        for g in range(1, 4):
            acc = acc + res.results[4 * b + g]["out"]
        outp[b] = acc.T
    return outp



# revision 5
# speedup vs baseline: 1.0301x; 1.0301x over previous
"""Multi-head attention (B=2, S=2048, D=1024, H=16, causal) on 8 TRN2 NeuronCores.

Sharding: 8 cores = 2 batches x 4 head-groups (4 heads each).  Each core
computes the QKV projections for its head slice, causal attention for its 4
heads, and the partial output projection (input-dim slice of Wo).  The
all-reduce over head-groups happens at gather time on the host (sum of 4
partials per batch), which is the "all-reduce after the output projection"
of a tensor-parallel split.

Everything on device works in token-transposed layout ([feature, token]) so
no on-device transposes are needed:
  scores^T[kv, q] = K_projT_tile^T @ Q_projT   (K = dh = 64)
  P = exp(scores^T)  (no max subtraction needed: scores ~ N(0,1), |s| < ~7)
  out^T[dh(+1), q] = [V | ones]^T @ P          (ones column -> softmax denom)
  partial^T[dmodel, tok] = WoT_slice^T @ attn_out^T

Perf structure (v2 schedule):
  - inputs are host-swizzled so DMA arrives in compute order: q/k in
    512-token q-tile slices [128, 8kt, 512], v in 128-token kv-tile slices
    [128, 8kt, 128]; the first projection group starts ~4us earlier.
  - warmup matmuls at t=0 keep the PE HAM activity monitor busy so the
    clock gate opens (1.2 -> 2.4 GHz) before real work lands.
  - all work (QKV projections, both head-pair attention chains, O
    projection) is interleaved at kv-tile granularity in one long tensor
    queue, so the PE never idles >3.4us and exp (scalar engine) overlaps
    matmul throughout.
  - exp for the two packed heads is issued as ONE activation over a
    2-bank PSUM tile ([128, 2, 512]) halving ACT instruction overhead.
  - softmax denominator: reciprocal reads the PSUM ones-row directly and
    gpsimd broadcasts it; no SBUF->SBUF DMA round-trip.
  - output partials are cast to bf16 before the store DMA (half traffic);
    host sums in f32.
"""

import math
import os

import numpy as np
import ml_dtypes

_BF16 = ml_dtypes.bfloat16

B, S, D = 2, 2048, 1024
H, DH = 16, 64
NCORES = 8
GRP = 4  # heads per core
KT = D // 128  # 8 k-tiles over d_model
NQ = 512  # q tile width (free dim of score tiles)
QTILES = S // NQ  # 4
KVTILES = S // 128  # 16

last_results = None

_programs = {}


def _build_program(causal: bool):
    WARM = int(os.environ.get("KOPT_WARM", "64"))

    import concourse.bass as bass
    import concourse.mybir as mybir
    import concourse.tile as tile
    from concourse import bacc

    f32 = mybir.dt.float32
    bf16 = mybir.dt.bfloat16
    Exp = mybir.ActivationFunctionType.Exp

    nc = bacc.Bacc(
        "TRN2",
        target_bir_lowering=False,
        debug=False,
        enable_asserts=False,
        num_devices=NCORES,
    )

    # host-swizzled inputs: qTs[p, n, kt, c] = q^T[128*kt+p, 512*n+c]
    qTs = nc.dram_tensor("qTs", [128, QTILES, KT, NQ], bf16, kind="ExternalInput").ap()
    kTs = nc.dram_tensor("kTs", [128, QTILES, KT, NQ], bf16, kind="ExternalInput").ap()
    # vTs[p, mt, kt, c] = v^T[128*kt+p, 128*mt+c]
    vTs = nc.dram_tensor("vTs", [128, KVTILES, KT, 128], bf16, kind="ExternalInput").ap()
    wqT = nc.dram_tensor("wqT", [D, 256], bf16, kind="ExternalInput").ap()
    wkT = nc.dram_tensor("wkT", [D, 256], bf16, kind="ExternalInput").ap()
    wvT = nc.dram_tensor("wvT", [D, 256], bf16, kind="ExternalInput").ap()
    woT = nc.dram_tensor("woT", [256, D], bf16, kind="ExternalInput").ap()
    if not causal:
        maskT = nc.dram_tensor("maskT", [S, S], bf16, kind="ExternalInput").ap()
    out = nc.dram_tensor("out", [D, S], bf16, kind="ExternalOutput").ap()

    with tile.TileContext(nc) as tc:
        with (
            tc.tile_pool(name="persist", bufs=1) as sb,
            tc.tile_pool(name="stream", bufs=3) as stream,
            tc.tile_pool(name="psum", bufs=1, space="PSUM") as psum,
            tc.tile_pool(name="p_sb", bufs=4) as pbuf,
            tc.tile_pool(name="r_sb", bufs=4) as rpool,
            tc.tile_pool(name="m_sb", bufs=4) as mpool,
            tc.tile_pool(name="o_sb", bufs=4) as opool,
        ):
            # ---- persistent SBUF tensors ----
            wq_sb = sb.tile([128, KT, 256], bf16)
            wk_sb = sb.tile([128, KT, 256], bf16)
            wv_sb = sb.tile([128, KT, 256], bf16)
            wo2 = sb.tile([128, 2, D], bf16)  # head h at rows 64*(h%2), chunk h//2
            qproj = sb.tile([128, 2, S], bf16)
            kproj = sb.tile([128, 2, S], bf16)
            vproj = sb.tile([128, KVTILES, GRP, 66], bf16)
            attn2 = sb.tile([128, 2, S], bf16)  # head h at rows 64*(h%2), chunk h//2

            # ---- warmup: keep the PE HAM window busy from t=0 so the
            # clock is at 2.4 GHz when the first real matmul lands ----
            if WARM:
                wz = sb.tile([128, 128], bf16)
                nc.gpsimd.memset(wz[:], 0.0)
                warm_ps = psum.tile([65, NQ], f32, tag="av", bufs=2, name="warm")
                for _ in range(WARM):
                    nc.tensor.matmul(
                        warm_ps[:, 0:128], wz[:, 0:65], wz[:], start=True, stop=True
                    )

            if causal:
                # single 128x128 causal block: keep where q_local >= kv_local
                mask128 = sb.tile([128, 128], bf16)
                nc.gpsimd.memset(mask128[:], 1.0)
                nc.gpsimd.affine_select(
                    out=mask128[:],
                    in_=mask128[:],
                    compare_op=mybir.AluOpType.is_ge,
                    fill=0.0,
                    base=0,
                    pattern=[[1, 128]],
                    channel_multiplier=-1,
                )

            # ones columns at index 0 and 65 of vproj (V lands in cols 1..64);
            # on the vector engine, which is otherwise idle until ~13us
            nc.vector.memset(vproj[:], 1.0)

            # ---- input DMAs (order = arrival order per queue) ----
            # sync queue:   wq | q n0..n3 | vmt 8..15
            # gpsimd queue: wk | k n0 | wv | vmt 0..3 | k n1 | wo | vmt 4..7 | k n2 | k n3
            qn, kn, vmt = {}, {}, {}

            def dma_qn(n):
                t = stream.tile([128, KT, NQ], bf16, tag="qn", bufs=3)
                nc.sync.dma_start(t[:], qTs[:, n, :, :])
                qn[n] = t

            def dma_kn(n):
                t = stream.tile([128, KT, NQ], bf16, tag="kn", bufs=3)
                nc.gpsimd.dma_start(t[:], kTs[:, n, :, :])
                kn[n] = t

            def dma_vmt(mt, queue):
                t = stream.tile([128, KT, 128], bf16, tag="vmt", bufs=6)
                queue.dma_start(t[:], vTs[:, mt, :, :])
                vmt[mt] = t

            for kt in range(KT):
                nc.sync.dma_start(wq_sb[:, kt, :], wqT[128 * kt : 128 * kt + 128, :])
            for n in range(QTILES):
                dma_qn(n)
            for kt in range(KT):
                nc.gpsimd.dma_start(wk_sb[:, kt, :], wkT[128 * kt : 128 * kt + 128, :])
            dma_kn(0)
            for kt in range(KT):
                nc.gpsimd.dma_start(wv_sb[:, kt, :], wvT[128 * kt : 128 * kt + 128, :])
            for mt in range(0, 4):
                dma_vmt(mt, nc.gpsimd)
            dma_kn(1)
            for h in range(GRP):
                base = 64 * (h % 2)
                nc.gpsimd.dma_start(
                    wo2[base : base + 64, h // 2, :], woT[64 * h : 64 * h + 64, :]
                )
            for mt in range(4, 8):
                dma_vmt(mt, nc.gpsimd)
            dma_kn(2)
            for mt in range(8, 12):
                dma_vmt(mt, nc.sync)
            dma_kn(3)
            for mt in range(12, 16):
                dma_vmt(mt, nc.sync)

            # ---- emit helpers ----
            def qkproj(which, m2, n):
                w_sb = wq_sb if which == "q" else wk_sb
                xt = qn[n] if which == "q" else kn[n]
                proj = qproj if which == "q" else kproj
                ps = psum.tile([128, NQ], f32, tag="op", bufs=2)
                for kt in range(KT):
                    nc.tensor.matmul(
                        ps[:],
                        w_sb[:, kt, 128 * m2 : 128 * m2 + 128],
                        xt[:, kt, :],
                        start=(kt == 0),
                        stop=(kt == KT - 1),
                    )
                nc.vector.tensor_copy(proj[:, m2, NQ * n : NQ * n + NQ], ps[:])

            def vproj_tile(mt):
                ps = psum.tile([128, 256], f32, tag="op", bufs=2)
                for kt in range(KT):
                    nc.tensor.matmul(
                        ps[:],
                        vmt[mt][:, kt, :],
                        wv_sb[:, kt, :],
                        start=(kt == 0),
                        stop=(kt == KT - 1),
                    )
                nc.vector.tensor_copy(
                    vproj[:, mt, :, 1:65],
                    ps[:].rearrange("p (h d) -> p h d", h=GRP),
                )

            # attention chain state per (c2): av accumulators created at t=0
            class Chain:
                pass

            def attn_start(c2, j):
                ch = Chain()
                ch.c2, ch.j = c2, j
                ch.avs = [
                    psum.tile([65, NQ], f32, tag="av", bufs=2, name=f"av{c2}{j}{i}")
                    for i in range(2)
                ]
                ch.ktiles = 4 * j + 4 if causal else KVTILES
                return ch

            def attn_step(ch, t):
                c2, j = ch.c2, ch.j
                d = t - 4 * j
                off = 128 * d if (causal and d >= 0) else 0
                # merged score psum: [128, 2, NQ] spans two banks
                sp = psum.tile([128, 2, NQ], f32, tag="sc", bufs=2)
                for i in range(2):
                    base = 64 * i
                    nc.tensor.matmul(
                        sp[:, i, off:NQ],
                        kproj[base : base + 64, c2, 128 * t : 128 * t + 128],
                        qproj[base : base + 64, c2, NQ * j + off : NQ * j + NQ],
                        start=True,
                        stop=True,
                    )
                p = pbuf.tile([128, 2, NQ], bf16, tag="p")
                nc.scalar.activation(p[:, :, off:NQ], sp[:, :, off:NQ], Exp)
                if causal:
                    if d >= 0:
                        for i in range(2):
                            nc.vector.tensor_mul(
                                p[:, i, off : off + 128],
                                p[:, i, off : off + 128],
                                mask128[:],
                            )
                else:
                    mt_t = mpool.tile([128, NQ], bf16, tag="mt")
                    nc.sync.dma_start(
                        mt_t[:],
                        maskT[128 * t : 128 * t + 128, NQ * j : NQ * j + NQ],
                    )
                    for i in range(2):
                        nc.vector.tensor_mul(p[:, i, :], p[:, i, :], mt_t[:])
                for i in range(2):
                    nc.tensor.matmul(
                        ch.avs[i][:, off:NQ],
                        vproj[:, t, 2 * c2 + i, 1:66],
                        p[:, i, off:NQ],
                        start=(t == 0),
                        stop=(t == ch.ktiles - 1),
                    )

            def attn_norm(ch):
                # attn2[rows, c2, q] = av[0:64, q] / av[64, q]
                c2, j = ch.c2, ch.j
                for i in range(2):
                    av = ch.avs[i]
                    rr = rpool.tile([1, NQ], f32, tag="rr")
                    nc.vector.reciprocal(rr[:], av[64:65, :])
                    rb = rpool.tile([64, NQ], f32, tag="rb")
                    nc.gpsimd.partition_broadcast(rb[:], rr[0:1, :], channels=64)
                    if i == 0:
                        nc.vector.tensor_mul(
                            attn2[0:64, c2, NQ * j : NQ * j + NQ], av[0:64, :], rb[:]
                        )
                    else:
                        tmpn = rpool.tile([64, NQ], bf16, tag="tmpn")
                        nc.vector.tensor_mul(tmpn[:], av[0:64, :], rb[:])
                        nc.sync.dma_start(
                            attn2[64:128, c2, NQ * j : NQ * j + NQ], tmpn[:]
                        )

            def oproj_m(n, m):
                ps = psum.tile([128, NQ], f32, tag="op", bufs=2)
                # head pairs stacked in partition halves -> K=128 contracts
                # two heads per matmul
                for c2 in range(2):
                    nc.tensor.matmul(
                        ps[:],
                        wo2[:, c2, 128 * m : 128 * m + 128],
                        attn2[:, c2, NQ * n : NQ * n + NQ],
                        start=(c2 == 0),
                        stop=(c2 == 1),
                    )
                ot = opool.tile([128, NQ], bf16, tag="ot")
                nc.vector.tensor_copy(ot[:], ps[:])
                nc.sync.dma_start(out[128 * m : 128 * m + 128, NQ * n : NQ * n + NQ], ot[:])

            # ---- global schedule ----
            # Fillers are emitted BETWEEN attention t-steps so the tensor
            # queue (strict in-order) always has independent matmul work
            # while exp/mask/AV dependencies resolve.
            def run_round(ch, fillers):
                """attention t-loop with filler thunks spread over steps."""
                nt = ch.ktiles
                nf = len(fillers)
                fi = 0
                for t in range(nt):
                    attn_step(ch, t)
                    # distribute fillers evenly across steps
                    want = (t + 1) * nf // nt
                    while fi < want:
                        fillers[fi]()
                        fi += 1
                while fi < nf:
                    fillers[fi]()
                    fi += 1

            F = lambda f, *a: (lambda: f(*a))

            # R0: initial projections (DMA-gated; queue them densely)
            qkproj("q", 0, 0)
            qkproj("q", 1, 0)
            qkproj("k", 0, 0)
            qkproj("k", 1, 0)
            for mt in range(0, 4):
                vproj_tile(mt)

            # j = 0
            ch0 = attn_start(0, 0)
            run_round(ch0, [F(qkproj, "q", 0, 1), F(qkproj, "q", 1, 1)])
            attn_norm(ch0)
            ch1 = attn_start(1, 0)
            run_round(ch1, [F(qkproj, "k", 0, 1), F(qkproj, "k", 1, 1)])
            attn_norm(ch1)

            # j = 1 c2=0 | vproj 4..7, qproj n2
            ch0 = attn_start(0, 1)
            run_round(
                ch0,
                [F(vproj_tile, 4), F(vproj_tile, 5), F(vproj_tile, 6), F(vproj_tile, 7),
                 F(qkproj, "q", 0, 2), F(qkproj, "q", 1, 2)],
            )
            attn_norm(ch0)
            # j = 1 c2=1 | oproj(0), kproj n2
            ch1 = attn_start(1, 1)
            run_round(
                ch1,
                [F(oproj_m, 0, m) for m in range(8)]
                + [F(qkproj, "k", 0, 2), F(qkproj, "k", 1, 2)],
            )
            attn_norm(ch1)

            # j = 2 c2=0 | vproj 8..11, qproj n3
            ch0 = attn_start(0, 2)
            run_round(
                ch0,
                [F(vproj_tile, 8), F(vproj_tile, 9), F(vproj_tile, 10), F(vproj_tile, 11),
                 F(qkproj, "q", 0, 3), F(qkproj, "q", 1, 3)],
            )
            attn_norm(ch0)
            # j = 2 c2=1 | oproj(1), kproj n3
            ch1 = attn_start(1, 2)
            run_round(
                ch1,
                [F(oproj_m, 1, m) for m in range(8)]
                + [F(qkproj, "k", 0, 3), F(qkproj, "k", 1, 3)],
            )
            attn_norm(ch1)

            # j = 3 c2=0 | vproj 12..15, oproj(2) first half
            ch0 = attn_start(0, 3)
            run_round(
                ch0,
                [F(vproj_tile, 12), F(vproj_tile, 13), F(vproj_tile, 14), F(vproj_tile, 15)]
                + [F(oproj_m, 2, m) for m in range(4)],
            )
            attn_norm(ch0)
            # j = 3 c2=1 | oproj(2) second half
            ch1 = attn_start(1, 3)
            run_round(ch1, [F(oproj_m, 2, m) for m in range(4, 8)])
            attn_norm(ch1)

            # tail: oproj(3)
            for m in range(8):
                oproj_m(3, m)

    nc.compile()
    return nc


def _get_program(causal: bool):
    if causal not in _programs:
        _programs[causal] = _build_program(causal)
    return _programs[causal]


def kernel(query, key, value, mask, Wq, Wk, Wv, Wo):
    global last_results
    from concourse.bass_utils import run_bass_kernel_spmd

    query = np.asarray(query, dtype=np.float32)
    key = np.asarray(key, dtype=np.float32)
    value = np.asarray(value, dtype=np.float32)
    Wq = np.asarray(Wq, dtype=np.float32)
    Wk = np.asarray(Wk, dtype=np.float32)
    Wv = np.asarray(Wv, dtype=np.float32)
    Wo = np.asarray(Wo, dtype=np.float32)
    m2d = np.asarray(mask).reshape(S, S).astype(bool)

    causal = bool(np.array_equal(m2d, np.tril(np.ones((S, S), dtype=bool))))
    nc = _get_program(causal)

    scale = 1.0 / math.sqrt(DH)
    WqT = np.ascontiguousarray((Wq * scale).T).astype(_BF16)
    WkT = np.ascontiguousarray(Wk.T).astype(_BF16)
    WvT = np.ascontiguousarray(Wv.T).astype(_BF16)
    WoT = np.ascontiguousarray(Wo.T).astype(_BF16)

    def swz_qk(x):  # [S, D] f32 -> [128, QTILES, KT, NQ] bf16
        xT = x.T  # [D, S]
        return np.ascontiguousarray(
            xT.reshape(KT, 128, QTILES, NQ).transpose(1, 2, 0, 3)
        ).astype(_BF16)

    def swz_v(x):  # [S, D] f32 -> [128, KVTILES, KT, 128] bf16
        xT = x.T
        return np.ascontiguousarray(
            xT.reshape(KT, 128, KVTILES, 128).transpose(1, 2, 0, 3)
        ).astype(_BF16)

    qs = [swz_qk(query[b]) for b in range(B)]
    ks = [swz_qk(key[b]) for b in range(B)]
    vs = [swz_v(value[b]) for b in range(B)]
    if not causal:
        maskTb = np.ascontiguousarray(m2d.T).astype(_BF16)

    in_maps = []
    for c in range(NCORES):
        b, g = c // 4, c % 4
        sl = slice(256 * g, 256 * g + 256)
        im = {
            "qTs": qs[b],
            "kTs": ks[b],
            "vTs": vs[b],
            "wqT": np.ascontiguousarray(WqT[:, sl]),
            "wkT": np.ascontiguousarray(WkT[:, sl]),
            "wvT": np.ascontiguousarray(WvT[:, sl]),
            "woT": np.ascontiguousarray(WoT[sl, :]),
        }
        if not causal:
            im["maskT"] = maskTb
        in_maps.append(im)

    trace = os.environ.get("KERNEL_PROFILE", "") == "1"
    res = run_bass_kernel_spmd(nc, in_maps, list(range(NCORES)), trace=trace)
    last_results = res

    outp = np.empty((B, S, D), dtype=np.float32)
    for b in range(B):
        acc = res.results[4 * b]["out"].astype(np.float32)
        for g in range(1, 4):
            acc = acc + res.results[4 * b + g]["out"].astype(np.float32)
        outp[b] = acc.T
    return outp


# revision 9
# speedup vs baseline: 1.3061x; 1.2679x over previous
"""Multi-head attention (B=2, S=2048, D=1024, H=16, causal) on 8 TRN2 NeuronCores.

Sharding: 8 cores = 2 batches x 4 head-groups (4 heads each).  Each core
computes the QKV projections for its head slice, causal attention for its 4
heads, and the partial output projection (input-dim slice of Wo).  The
all-reduce over head-groups happens at gather time on the host (sum of 4
partials per batch), which is the "all-reduce after the output projection"
of a tensor-parallel split.

Everything on device works in token-transposed layout ([feature, token]) so
no on-device transposes are needed:
  scores^T[kv, q] = K_projT_tile^T @ Q_projT   (K = dh = 64)
  P = exp(scores^T)  (no max subtraction needed: scores ~ N(0,1), |s| < ~7)
  out^T[dh(+1), q] = [V | ones]^T @ P          (ones column -> softmax denom)
  partial^T[dmodel, tok] = WoT_slice^T @ attn_out^T

Perf structure (v2 schedule):
  - inputs are host-swizzled so DMA arrives in compute order: q/k in
    512-token q-tile slices [128, 8kt, 512], v in 128-token kv-tile slices
    [128, 8kt, 128]; the first projection group starts ~4us earlier.
  - warmup matmuls at t=0 keep the PE HAM activity monitor busy so the
    clock gate opens (1.2 -> 2.4 GHz) before real work lands.
  - all work (QKV projections, both head-pair attention chains, O
    projection) is interleaved at kv-tile granularity in one long tensor
    queue, so the PE never idles >3.4us and exp (scalar engine) overlaps
    matmul throughout.
  - exp for the two packed heads is issued as ONE activation over a
    2-bank PSUM tile ([128, 2, 512]) halving ACT instruction overhead.
  - softmax denominator: reciprocal reads the PSUM ones-row directly and
    gpsimd broadcasts it; no SBUF->SBUF DMA round-trip.
  - output partials are cast to bf16 before the store DMA (half traffic);
    host sums in f32.
"""

import math
import os

import numpy as np
import ml_dtypes

_BF16 = ml_dtypes.bfloat16

B, S, D = 2, 2048, 1024
H, DH = 16, 64
NCORES = 8
GRP = 4  # heads per core
KT = D // 128  # 8 k-tiles over d_model
NQ = 512  # q tile width (free dim of score tiles)
QTILES = S // NQ  # 4
KVTILES = S // 128  # 16

last_results = None

_programs = {}


def _build_program(causal: bool):
    WARM = int(os.environ.get("KOPT_WARM", "64"))

    import concourse.bass as bass
    import concourse.mybir as mybir
    import concourse.tile as tile
    from concourse import bacc

    f32 = mybir.dt.float32
    bf16 = mybir.dt.bfloat16
    Exp = mybir.ActivationFunctionType.Exp

    nc = bacc.Bacc(
        "TRN2",
        target_bir_lowering=False,
        debug=False,
        enable_asserts=False,
        num_devices=NCORES,
    )

    # host-swizzled inputs: qTs[p, n, kt, c] = q^T[128*kt+p, 512*n+c]
    qTs = nc.dram_tensor("qTs", [128, QTILES, KT, NQ], bf16, kind="ExternalInput").ap()
    kTs = nc.dram_tensor("kTs", [128, QTILES, KT, NQ], bf16, kind="ExternalInput").ap()
    # vTs[p, mt, kt, c] = v^T[128*kt+p, 128*mt+c]
    vTs = nc.dram_tensor("vTs", [128, KVTILES, KT, 128], bf16, kind="ExternalInput").ap()
    wqT = nc.dram_tensor("wqT", [D, 256], bf16, kind="ExternalInput").ap()
    wkT = nc.dram_tensor("wkT", [D, 256], bf16, kind="ExternalInput").ap()
    wvT = nc.dram_tensor("wvT", [D, 256], bf16, kind="ExternalInput").ap()
    woT = nc.dram_tensor("woT", [256, D], bf16, kind="ExternalInput").ap()
    if not causal:
        maskT = nc.dram_tensor("maskT", [S, S], bf16, kind="ExternalInput").ap()
    out = nc.dram_tensor("out", [D, S], bf16, kind="ExternalOutput").ap()

    with tile.TileContext(nc) as tc:
        with (
            tc.tile_pool(name="persist", bufs=1) as sb,
            tc.tile_pool(name="stream", bufs=3) as stream,
            tc.tile_pool(name="psum", bufs=1, space="PSUM") as psum,
            tc.tile_pool(name="p_sb", bufs=4) as pbuf,
            tc.tile_pool(name="r_sb", bufs=4) as rpool,
            tc.tile_pool(name="m_sb", bufs=4) as mpool,
            tc.tile_pool(name="o_sb", bufs=4) as opool,
        ):
            # ---- persistent SBUF tensors ----
            wq_sb = sb.tile([128, KT, 256], bf16)
            wk_sb = sb.tile([128, KT, 256], bf16)
            wv_sb = sb.tile([128, KT, 256], bf16)
            wo2 = sb.tile([128, 2, D], bf16)  # head h at rows 64*(h%2), chunk h//2
            qproj = sb.tile([128, 2, S], bf16)
            kproj = sb.tile([128, 2, S], bf16)
            vproj = sb.tile([128, KVTILES, GRP, 66], bf16)
            attn2 = sb.tile([128, 2, S], bf16)  # head h at rows 64*(h%2), chunk h//2

            # ---- warmup: keep the PE HAM window busy from t=0 so the
            # clock is at 2.4 GHz when the first real matmul lands ----
            if WARM:
                wz = sb.tile([128, 128], bf16)
                nc.gpsimd.memset(wz[:], 0.0)
                warm_ps = psum.tile([65, NQ], f32, tag="av", bufs=2, name="warm")
                for _ in range(WARM):
                    nc.tensor.matmul(
                        warm_ps[:, 0:128], wz[:, 0:65], wz[:], start=True, stop=True
                    )

            if causal:
                # single 128x128 causal block: keep where q_local >= kv_local
                mask128 = sb.tile([128, 128], bf16)
                nc.gpsimd.memset(mask128[:], 1.0)
                nc.gpsimd.affine_select(
                    out=mask128[:],
                    in_=mask128[:],
                    compare_op=mybir.AluOpType.is_ge,
                    fill=0.0,
                    base=0,
                    pattern=[[1, 128]],
                    channel_multiplier=-1,
                )

            # ones columns at index 0 and 65 of vproj (V lands in cols 1..64);
            # on the vector engine, which is otherwise idle until ~13us
            nc.vector.memset(vproj[:], 1.0)

            # ---- input DMAs (order = arrival order per queue) ----
            # sync queue:   wq | q n0..n3 | vmt 8..15
            # gpsimd queue: wk | k n0 | wv | vmt 0..3 | k n1 | wo | vmt 4..7 | k n2 | k n3
            qn, kn, vmt = {}, {}, {}

            def dma_qn(n):
                t = stream.tile([128, KT, NQ], bf16, tag="qn", bufs=3)
                nc.sync.dma_start(t[:], qTs[:, n, :, :])
                qn[n] = t

            def dma_kn(n):
                t = stream.tile([128, KT, NQ], bf16, tag="kn", bufs=3)
                nc.gpsimd.dma_start(t[:], kTs[:, n, :, :])
                kn[n] = t

            def dma_vmt(mt, queue):
                t = stream.tile([128, KT, 128], bf16, tag="vmt", bufs=6)
                queue.dma_start(t[:], vTs[:, mt, :, :])
                vmt[mt] = t

            # interleave weight-tile and first-q-tile DMAs so matmul kt's
            # two dependencies (wq[kt], qn0[kt]) arrive back to back
            q0 = stream.tile([128, KT, NQ], bf16, tag="qn", bufs=3)
            for kt in range(KT):
                nc.sync.dma_start(wq_sb[:, kt, :], wqT[128 * kt : 128 * kt + 128, :])
                nc.sync.dma_start(q0[:, kt, :], qTs[:, 0, kt, :])
            qn[0] = q0
            for n in range(1, QTILES):
                dma_qn(n)
            k0 = stream.tile([128, KT, NQ], bf16, tag="kn", bufs=3)
            for kt in range(KT):
                nc.gpsimd.dma_start(wk_sb[:, kt, :], wkT[128 * kt : 128 * kt + 128, :])
                nc.gpsimd.dma_start(k0[:, kt, :], kTs[:, 0, kt, :])
            kn[0] = k0
            for kt in range(KT):
                nc.gpsimd.dma_start(wv_sb[:, kt, :], wvT[128 * kt : 128 * kt + 128, :])
            for mt in range(0, 4):
                dma_vmt(mt, nc.gpsimd)
            dma_kn(1)
            for h in range(GRP):
                base = 64 * (h % 2)
                nc.gpsimd.dma_start(
                    wo2[base : base + 64, h // 2, :], woT[64 * h : 64 * h + 64, :]
                )
            for mt in range(4, 8):
                dma_vmt(mt, nc.gpsimd)
            dma_kn(2)
            for mt in range(8, 12):
                dma_vmt(mt, nc.sync)
            dma_kn(3)
            for mt in range(12, 16):
                dma_vmt(mt, nc.sync)

            # ---- emit helpers ----
            def qkproj(which, m2, n):
                w_sb = wq_sb if which == "q" else wk_sb
                xt = qn[n] if which == "q" else kn[n]
                proj = qproj if which == "q" else kproj
                ps = psum.tile([128, NQ], f32, tag="op", bufs=2)
                for kt in range(KT):
                    nc.tensor.matmul(
                        ps[:],
                        w_sb[:, kt, 128 * m2 : 128 * m2 + 128],
                        xt[:, kt, :],
                        start=(kt == 0),
                        stop=(kt == KT - 1),
                    )
                nc.vector.tensor_copy(proj[:, m2, NQ * n : NQ * n + NQ], ps[:])

            def vproj_tile(mt):
                ps = psum.tile([128, 256], f32, tag="op", bufs=2)
                for kt in range(KT):
                    nc.tensor.matmul(
                        ps[:],
                        vmt[mt][:, kt, :],
                        wv_sb[:, kt, :],
                        start=(kt == 0),
                        stop=(kt == KT - 1),
                    )
                nc.vector.tensor_copy(
                    vproj[:, mt, :, 1:65],
                    ps[:].rearrange("p (h d) -> p h d", h=GRP),
                )

            # attention chain state per (c2): av accumulators created at t=0
            class Chain:
                pass

            def attn_start(c2, j):
                ch = Chain()
                ch.c2, ch.j = c2, j
                ch.avs = [
                    psum.tile([65, NQ], f32, tag="av", bufs=2, name=f"av{c2}{j}{i}")
                    for i in range(2)
                ]
                ch.ktiles = 4 * j + 4 if causal else KVTILES
                return ch

            def attn_step(ch, t):
                c2, j = ch.c2, ch.j
                d = t - 4 * j
                off = 128 * d if (causal and d >= 0) else 0
                # merged score psum: [128, 2, NQ] spans two banks
                sp = psum.tile([128, 2, NQ], f32, tag="sc", bufs=2)
                for i in range(2):
                    base = 64 * i
                    nc.tensor.matmul(
                        sp[:, i, off:NQ],
                        kproj[base : base + 64, c2, 128 * t : 128 * t + 128],
                        qproj[base : base + 64, c2, NQ * j + off : NQ * j + NQ],
                        start=True,
                        stop=True,
                    )
                p = pbuf.tile([128, 2, NQ], bf16, tag="p")
                nc.scalar.activation(p[:, :, off:NQ], sp[:, :, off:NQ], Exp)
                if causal:
                    if d >= 0:
                        for i in range(2):
                            nc.vector.tensor_mul(
                                p[:, i, off : off + 128],
                                p[:, i, off : off + 128],
                                mask128[:],
                            )
                else:
                    mt_t = mpool.tile([128, NQ], bf16, tag="mt")
                    nc.sync.dma_start(
                        mt_t[:],
                        maskT[128 * t : 128 * t + 128, NQ * j : NQ * j + NQ],
                    )
                    for i in range(2):
                        nc.vector.tensor_mul(p[:, i, :], p[:, i, :], mt_t[:])
                for i in range(2):
                    nc.tensor.matmul(
                        ch.avs[i][:, off:NQ],
                        vproj[:, t, 2 * c2 + i, 1:66],
                        p[:, i, off:NQ],
                        start=(t == 0),
                        stop=(t == ch.ktiles - 1),
                    )

            def attn_norm(ch):
                # attn2[rows, c2, q] = av[0:64, q] / av[64, q].
                # Copy av to SBUF first: frees the PSUM bank for the next
                # chain immediately; the recip chain (DMA partition-spread
                # so reciprocal runs 128-wide) then runs off-critical-path.
                c2, j = ch.c2, ch.j
                avcs = []
                for i in range(2):
                    avc = mpool.tile([65, NQ], f32, tag="avc", bufs=4)
                    nc.vector.tensor_copy(avc[:], ch.avs[i][:])
                    avcs.append(avc)
                for i in range(2):
                    avc = avcs[i]
                    rq = rpool.tile([128, 4], f32, tag="rq")
                    nc.sync.dma_start(rq[:], avc[64:65, :])
                    rqr = rpool.tile([128, 4], f32, tag="rqr")
                    nc.vector.reciprocal(rqr[:], rq[:])
                    rr = rpool.tile([1, NQ], f32, tag="rr")
                    nc.sync.dma_start(rr[:], rqr[:])
                    rb = rpool.tile([64, NQ], f32, tag="rb")
                    nc.gpsimd.partition_broadcast(rb[:], rr[0:1, :], channels=64)
                    if i == 0:
                        nc.vector.tensor_mul(
                            attn2[0:64, c2, NQ * j : NQ * j + NQ], avc[0:64, :], rb[:]
                        )
                    else:
                        tmpn = rpool.tile([64, NQ], bf16, tag="tmpn")
                        nc.vector.tensor_mul(tmpn[:], avc[0:64, :], rb[:])
                        nc.sync.dma_start(
                            attn2[64:128, c2, NQ * j : NQ * j + NQ], tmpn[:]
                        )

            def oproj_m(n, m):
                ps = psum.tile([128, NQ], f32, tag="op", bufs=2)
                # head pairs stacked in partition halves -> K=128 contracts
                # two heads per matmul
                for c2 in range(2):
                    nc.tensor.matmul(
                        ps[:],
                        wo2[:, c2, 128 * m : 128 * m + 128],
                        attn2[:, c2, NQ * n : NQ * n + NQ],
                        start=(c2 == 0),
                        stop=(c2 == 1),
                    )
                ot = opool.tile([128, NQ], bf16, tag="ot")
                nc.vector.tensor_copy(ot[:], ps[:])
                nc.sync.dma_start(out[128 * m : 128 * m + 128, NQ * n : NQ * n + NQ], ot[:])

            # ---- global schedule ----
            # Fillers are emitted BETWEEN attention t-steps so the tensor
            # queue (strict in-order) always has independent matmul work
            # while exp/mask/AV dependencies resolve.
            def run_round(ch, fillers):
                """attention t-loop with filler thunks spread over steps."""
                nt = ch.ktiles
                nf = len(fillers)
                fi = 0
                for t in range(nt):
                    attn_step(ch, t)
                    # distribute fillers evenly across steps
                    want = (t + 1) * nf // nt
                    while fi < want:
                        fillers[fi]()
                        fi += 1
                while fi < nf:
                    fillers[fi]()
                    fi += 1

            F = lambda f, *a: (lambda: f(*a))

            # R0: initial projections (DMA-gated; queue them densely)
            qkproj("q", 0, 0)
            qkproj("q", 1, 0)
            qkproj("k", 0, 0)
            qkproj("k", 1, 0)
            for mt in range(0, 4):
                vproj_tile(mt)

            # j = 0
            ch0 = attn_start(0, 0)
            run_round(ch0, [F(qkproj, "q", 0, 1), F(qkproj, "q", 1, 1)])
            attn_norm(ch0)
            ch1 = attn_start(1, 0)
            run_round(ch1, [F(qkproj, "k", 0, 1), F(qkproj, "k", 1, 1)])
            attn_norm(ch1)

            # j = 1 c2=0 | vproj 4..7, qproj n2
            ch0 = attn_start(0, 1)
            run_round(
                ch0,
                [F(vproj_tile, 4), F(vproj_tile, 5), F(vproj_tile, 6), F(vproj_tile, 7),
                 F(qkproj, "q", 0, 2), F(qkproj, "q", 1, 2)],
            )
            attn_norm(ch0)
            # j = 1 c2=1 | oproj(0), kproj n2
            ch1 = attn_start(1, 1)
            run_round(
                ch1,
                [F(oproj_m, 0, m) for m in range(8)]
                + [F(qkproj, "k", 0, 2), F(qkproj, "k", 1, 2)],
            )
            attn_norm(ch1)

            # j = 2 c2=0 | vproj 8..11, qproj n3
            ch0 = attn_start(0, 2)
            run_round(
                ch0,
                [F(vproj_tile, 8), F(vproj_tile, 9), F(vproj_tile, 10), F(vproj_tile, 11),
                 F(qkproj, "q", 0, 3), F(qkproj, "q", 1, 3)],
            )
            attn_norm(ch0)
            # j = 2 c2=1 | oproj(1), kproj n3
            ch1 = attn_start(1, 2)
            run_round(
                ch1,
                [F(oproj_m, 1, m) for m in range(8)]
                + [F(qkproj, "k", 0, 3), F(qkproj, "k", 1, 3)],
            )
            attn_norm(ch1)

            # j = 3 c2=0 | vproj 12..15, oproj(2) first half
            ch0 = attn_start(0, 3)
            run_round(
                ch0,
                [F(vproj_tile, 12), F(vproj_tile, 13), F(vproj_tile, 14), F(vproj_tile, 15)]
                + [F(oproj_m, 2, m) for m in range(4)],
            )
            attn_norm(ch0)
            # j = 3 c2=1 | oproj(2) second half
            ch1 = attn_start(1, 3)
            run_round(ch1, [F(oproj_m, 2, m) for m in range(4, 8)])
            attn_norm(ch1)

            # tail: oproj(3)
            for m in range(8):
                oproj_m(3, m)

    nc.compile()
    return nc


def _get_program(causal: bool):
    if causal not in _programs:
        _programs[causal] = _build_program(causal)
    return _programs[causal]


def kernel(query, key, value, mask, Wq, Wk, Wv, Wo):
    global last_results
    from concourse.bass_utils import run_bass_kernel_spmd

    query = np.asarray(query, dtype=np.float32)
    key = np.asarray(key, dtype=np.float32)
    value = np.asarray(value, dtype=np.float32)
    Wq = np.asarray(Wq, dtype=np.float32)
    Wk = np.asarray(Wk, dtype=np.float32)
    Wv = np.asarray(Wv, dtype=np.float32)
    Wo = np.asarray(Wo, dtype=np.float32)
    m2d = np.asarray(mask).reshape(S, S).astype(bool)

    causal = bool(np.array_equal(m2d, np.tril(np.ones((S, S), dtype=bool))))
    nc = _get_program(causal)

    scale = 1.0 / math.sqrt(DH)
    WqT = np.ascontiguousarray((Wq * scale).T).astype(_BF16)
    WkT = np.ascontiguousarray(Wk.T).astype(_BF16)
    WvT = np.ascontiguousarray(Wv.T).astype(_BF16)
    WoT = np.ascontiguousarray(Wo.T).astype(_BF16)

    def swz_qk(x):  # [S, D] f32 -> [128, QTILES, KT, NQ] bf16
        xT = x.T  # [D, S]
        return np.ascontiguousarray(
            xT.reshape(KT, 128, QTILES, NQ).transpose(1, 2, 0, 3)
        ).astype(_BF16)

    def swz_v(x):  # [S, D] f32 -> [128, KVTILES, KT, 128] bf16
        xT = x.T
        return np.ascontiguousarray(
            xT.reshape(KT, 128, KVTILES, 128).transpose(1, 2, 0, 3)
        ).astype(_BF16)

    qs = [swz_qk(query[b]) for b in range(B)]
    ks = [swz_qk(key[b]) for b in range(B)]
    vs = [swz_v(value[b]) for b in range(B)]
    if not causal:
        maskTb = np.ascontiguousarray(m2d.T).astype(_BF16)

    in_maps = []
    for c in range(NCORES):
        b, g = c // 4, c % 4
        sl = slice(256 * g, 256 * g + 256)
        im = {
            "qTs": qs[b],
            "kTs": ks[b],
            "vTs": vs[b],
            "wqT": np.ascontiguousarray(WqT[:, sl]),
            "wkT": np.ascontiguousarray(WkT[:, sl]),
            "wvT": np.ascontiguousarray(WvT[:, sl]),
            "woT": np.ascontiguousarray(WoT[sl, :]),
        }
        if not causal:
            im["maskT"] = maskTb
        in_maps.append(im)

    trace = os.environ.get("KERNEL_PROFILE", "") == "1"
    res = run_bass_kernel_spmd(nc, in_maps, list(range(NCORES)), trace=trace)
    last_results = res

    outp = np.empty((B, S, D), dtype=np.float32)
    for b in range(B):
        acc = res.results[4 * b]["out"].astype(np.float32)
        for g in range(1, 4):
            acc = acc + res.results[4 * b + g]["out"].astype(np.float32)
        outp[b] = acc.T
    return outp
